# revision 21
# baseline (speedup 1.0000x reference)
"""AttentionDecoder kernel for 8 TRN2 NeuronCores.

Strategy (vocab-tensor-parallel, zero collectives):
  The GRU/attention part is tiny and latency-bound, so every core computes it
  redundantly (replicated) for the full batch; the heavy vocab projection
  (logits = cat @ w_out.T + b_out, 524 MB of output) is sharded over the vocab
  dimension: core c computes logits[:, :, c*4000:(c+1)*4000].  The host slices
  w_out/b_out per core and concatenates the logits shards; state/attention come
  from core 0.

Shapes (hardcoded): B=32, S_dec=128, S_enc=128, V=32000, E=200, H=128.
"""

import numpy as np

import concourse.bass as bass
from concourse import bacc
import concourse.mybir as mybir
import concourse.tile as tile
from concourse.bass import IndirectOffsetOnAxis
from concourse.bass_utils import run_bass_kernel_spmd
from concourse.masks import make_identity

B = 32
S = 128          # decoder length
SE = 128         # encoder length
EMB = 200        # embedding dim
H = 128          # hidden
V = 32000
NCORES = 8
VS = V // NCORES  # vocab shard = 4000
TOKS = B * S      # 4096, token index j = t*B + b  (time-major)

F32 = mybir.dt.float32
BF16 = mybir.dt.bfloat16
I32 = mybir.dt.int32

NB = 4            # decoder blocks
BT = S // NB      # timesteps per block = 32
NV = 8            # vocab chunks per core
VC = VS // NV     # 500 columns per chunk


def build_nc():
    nc = bacc.Bacc()

    # ---- I/O -------------------------------------------------------------
    seq_t = nc.declare_dram_parameter("seq_t", [TOKS], I32, isOutput=False)
    hs_d = nc.declare_dram_parameter("hs", [B, SE, H], F32, isOutput=False)
    h0_d = nc.declare_dram_parameter("h0", [B, H], F32, isOutput=False)
    emb_d = nc.declare_dram_parameter("emb", [V, EMB], F32, isOutput=False)
    w_ih_d = nc.declare_dram_parameter("w_ih", [3 * H, EMB], F32, isOutput=False)
    w_hh_d = nc.declare_dram_parameter("w_hh", [3 * H, H], F32, isOutput=False)
    b_ih_d = nc.declare_dram_parameter("b_ih", [3 * H], F32, isOutput=False)
    b_hh_d = nc.declare_dram_parameter("b_hh", [3 * H], F32, isOutput=False)
    w_outT_d = nc.declare_dram_parameter("w_outT", [2 * H, VS], F32, isOutput=False)
    b_out_d = nc.declare_dram_parameter("b_out", [VS], F32, isOutput=False)

    logits_d = nc.declare_dram_parameter("logits", [S, B, VS], F32, isOutput=True)
    state_d = nc.declare_dram_parameter("state", [B, H], F32, isOutput=True)
    aw_d = nc.declare_dram_parameter("aw", [B, SE, S], F32, isOutput=True)

    with tile.TileContext(nc) as tc:
        import contextlib
        ctx = contextlib.ExitStack()
        with ctx:
            persist = ctx.enter_context(tc.tile_pool(name="persist", bufs=1))
            work = ctx.enter_context(tc.tile_pool(name="work", bufs=3))
            stagep = ctx.enter_context(tc.tile_pool(name="stagep", bufs=4))
            grup = ctx.enter_context(tc.tile_pool(name="grup", bufs=3))
            psum_s = ctx.enter_context(
                tc.tile_pool(name="psum_s", bufs=4, space="PSUM"))
            psum_l = ctx.enter_context(
                tc.tile_pool(name="psum_l", bufs=4, space="PSUM"))

            # ---- persistent SBUF tensors --------------------------------
            ident_f = persist.tile([128, 128], F32)
            make_identity(nc, ident_f)

            xgT_rz = persist.tile([128, 2 * TOKS], F32)  # col = t*64 + g*32 + b
            xgT_n = persist.tile([128, TOKS], F32)       # col = t*32 + b
            outT = persist.tile([128, TOKS], F32)        # GRU outputs^T, col=t*32+b
            outT_bf = persist.tile([128, TOKS], BF16)
            cT_bf = persist.tile([128, TOKS], BF16)      # context^T, col=t*32+b
            hsT = persist.tile([128, B * SE], F32)       # hs[b]^T, col = b*128+e
            hs_bf = persist.tile([128, B * SE], BF16)    # hs[b],  col = b*128+h
            w_outT_bf = [persist.tile([128, VS], BF16, name=f"w_outT_bf{k}")
                         for k in range(2)]
            bcast_b = persist.tile([128, VS], BF16)      # b_out broadcast to rows
            hT0 = persist.tile([128, B], F32)
            ones_bf = persist.tile([1, 128], BF16)
            nc.gpsimd.memset(ones_bf[:], 1.0)

            w_ihT = [[persist.tile([128, 128], F32, name=f"w_ihT_{g}_{k}")
                      for k in range(2)] for g in range(3)]
            w_hhT = [persist.tile([128, 128], F32, name=f"w_hhT_{g}")
                     for g in range(3)]
            bias_r = persist.tile([128, 1], F32)
            bias_z = persist.tile([128, 1], F32)
            bih3 = persist.tile([128, 3], F32)
            bhh3 = persist.tile([128, 3], F32)
            bih3c = persist.tile([128, 3], F32)
            seq_sb = persist.tile([128, TOKS // 128], I32)

            # ---- P0: constants ------------------------------------------
            nc.sync.dma_start(seq_sb[:], seq_t.rearrange("(n p) -> p n", p=128))

            # biases: one DMA per tensor; each compute op waits on one queue
            nc.sync.dma_start(bih3[:], b_ih_d.rearrange("(g p) -> p g", p=128))
            nc.sync.dma_start(bhh3[:], b_hh_d.rearrange("(g p) -> p g", p=128))
            nc.vector.tensor_copy(bih3c[:], bih3[:])
            nc.vector.tensor_add(bias_r[:], bih3c[:, 0:1], bhh3[:, 0:1])
            nc.vector.tensor_add(bias_z[:], bih3c[:, 1:2], bhh3[:, 1:2])
            bias_nx = bih3c[:, 2:3]
            bias_nh = bhh3[:, 2:3]

            # w_ih -> w_ihT (bf16), w_hh -> w_hhT (f32)
            for g in range(3):
                wt = work.tile([128, EMB], F32, tag="wt")
                nc.sync.dma_start(wt[:], w_ih_d[g * 128:(g + 1) * 128, :])
                for k in range(2):
                    kn = 128 if k == 0 else EMB - 128
                    pt = psum_s.tile([128, 128], F32, tag="ps")
                    nc.tensor.transpose(
                        pt[:kn, :], wt[:, k * 128:k * 128 + kn], ident_f[:])
                    nc.vector.tensor_copy(w_ihT[g][k][:kn, :], pt[:kn, :])

                wh = work.tile([128, H], F32, tag="wh")
                nc.sync.dma_start(wh[:], w_hh_d[g * 128:(g + 1) * 128, :])
                ph = psum_s.tile([128, 128], F32, tag="ps")
                nc.tensor.transpose(ph[:], wh[:], ident_f[:])
                nc.vector.tensor_copy(w_hhT[g][:], ph[:])

            # h0 -> hT0
            h0t = work.tile([B, H], F32, tag="h0t")
            nc.sync.dma_start(h0t[:], h0_d[:])
            ph0 = psum_s.tile([128, 128], F32, tag="ps")
            nc.tensor.transpose(ph0[:, :B], h0t[:], ident_f[:B, :B])
            nc.vector.tensor_copy(hT0[:], ph0[:, :B])

            # hs: natural (bf16) + transposed (f32)
            for b in range(B):
                ht = work.tile([SE, H], F32, tag="ht")
                nc.sync.dma_start(ht[:], hs_d[b])
                nc.vector.tensor_copy(hs_bf[:, b * H:(b + 1) * H], ht[:])
                pht = psum_s.tile([128, 128], F32, tag="ps")
                nc.tensor.transpose(pht[:], ht[:], ident_f[:])
                nc.scalar.copy(hsT[:, b * SE:(b + 1) * SE], pht[:])

            # w_outT -> bf16; b_out -> broadcast rows (via ones matmul)
            bo = work.tile([1, VS], F32, tag="bo", bufs=1)
            nc.sync.dma_start(bo[:], b_out_d[None, :])
            bo_bf = work.tile([1, VS], BF16, tag="bo_bf", bufs=1)
            nc.vector.tensor_copy(bo_bf[:], bo[:])
            for k in range(2):
                for n in range(NV):
                    wo = stagep.tile([128, VC], F32, tag="wo", bufs=2)
                    nc.sync.dma_start(
                        wo[:], w_outT_d[k * 128:(k + 1) * 128,
                                        n * VC:(n + 1) * VC])
                    nc.vector.tensor_copy(
                        w_outT_bf[k][:, n * VC:(n + 1) * VC], wo[:])
            for n in range(NV):
                pb = psum_l.tile([128, VC], F32, tag="pl")
                nc.tensor.matmul(pb[:], ones_bf[0:1, :], bo_bf[0:1, n * VC:(n + 1) * VC],
                                 start=True, stop=True)
                nc.scalar.copy(bcast_b[:, n * VC:(n + 1) * VC], pb[:])

            # ---- P1+P2: embedding gather -> x^T (f32) -> x_gates^T ------
            xg_rz_4d = xgT_rz.rearrange("p (t g b) -> p t g b", g=2, b=B)
            for j in range(TOKS // 512):
                xt0 = work.tile([128, 512], F32, tag="xt0", bufs=2)
                xt1 = work.tile([128, 512], F32, tag="xt1", bufs=2)
                for i in range(4):
                    n = j * 4 + i
                    xr = work.tile([128, EMB], F32, tag="xr")
                    nc.gpsimd.indirect_dma_start(
                        out=xr[:], out_offset=None, in_=emb_d[:],
                        in_offset=IndirectOffsetOnAxis(
                            ap=seq_sb[:, n:n + 1], axis=0))
                    pt0 = psum_s.tile([128, 128], F32, tag="ps")
                    nc.tensor.transpose(pt0[:], xr[:, 0:128], ident_f[:])
                    nc.scalar.copy(xt0[:, i * 128:(i + 1) * 128], pt0[:])
                    pt1 = psum_s.tile([128, 128], F32, tag="ps")
                    nc.tensor.transpose(pt1[:EMB - 128, :], xr[:, 128:EMB],
                                        ident_f[:])
                    nc.scalar.copy(xt1[:EMB - 128, i * 128:(i + 1) * 128],
                                   pt1[:EMB - 128, :])
                t0 = j * 16   # 512 toks = 16 timesteps
                for g in range(3):
                    pxg = psum_s.tile([128, 512], F32, tag="ps")
                    nc.tensor.matmul(pxg[:], w_ihT[g][0][:], xt0[:],
                                     start=True, stop=False)
                    nc.tensor.matmul(pxg[:], w_ihT[g][1][:EMB - 128, :],
                                     xt1[:EMB - 128, :],
                                     start=False, stop=True)
                    if g == 0:
                        dst = xg_rz_4d[:, t0:t0 + 16, 0, :]
                        bias = bias_r
                    elif g == 1:
                        dst = xg_rz_4d[:, t0:t0 + 16, 1, :]
                        bias = bias_z
                    else:
                        dst = xgT_n[:, j * 512:(j + 1) * 512]
                        bias = bias_nx
                    nc.scalar.activation(
                        dst, pxg[:], mybir.ActivationFunctionType.Identity,
                        bias=bias[:, 0:1], scale=1.0)

            # ---- P3: GRU scan -------------------------------------------
            for t in range(S):
                hprev = hT0[:] if t == 0 else outT[:, (t - 1) * B:t * B]
                prz = psum_s.tile([128, 2 * B], F32, tag="ps")
                pn = psum_s.tile([128, B], F32, tag="ps")
                nc.tensor.matmul(prz[:, 0:B], w_hhT[0][:], hprev,
                                 start=True, stop=True)
                nc.tensor.matmul(prz[:, B:2 * B], w_hhT[1][:], hprev,
                                 start=True, stop=True)
                nc.tensor.matmul(pn[:], w_hhT[2][:], hprev, start=True, stop=True)

                rz_pre = grup.tile([128, 2 * B], F32, tag="rz_pre")
                nc.vector.tensor_add(rz_pre[:], xgT_rz[:, t * 2 * B:(t + 1) * 2 * B],
                                     prz[:])
                rz_sig = grup.tile([128, 2 * B], F32, tag="rz_sig")
                nc.scalar.activation(rz_sig[:], rz_pre[:],
                                     mybir.ActivationFunctionType.Sigmoid)
                hn_b = grup.tile([128, B], F32, tag="hn_b")
                nc.scalar.activation(hn_b[:], pn[:],
                                     mybir.ActivationFunctionType.Identity,
                                     bias=bias_nh[:, 0:1], scale=1.0)
                nr = grup.tile([128, B], F32, tag="nr")
                nc.vector.tensor_mul(nr[:], rz_sig[:, 0:B], hn_b[:])
                npre = grup.tile([128, B], F32, tag="npre")
                nc.vector.tensor_add(npre[:], nr[:], xgT_n[:, t * B:(t + 1) * B])
                n_t = grup.tile([128, B], F32, tag="n_t")
                nc.scalar.activation(n_t[:], npre[:],
                                     mybir.ActivationFunctionType.Tanh)
                d_t = grup.tile([128, B], F32, tag="d_t")
                nc.vector.tensor_sub(d_t[:], hprev, n_t[:])
                zd = grup.tile([128, B], F32, tag="zd")
                nc.vector.tensor_mul(zd[:], rz_sig[:, B:2 * B], d_t[:])
                nc.vector.tensor_add(outT[:, t * B:(t + 1) * B], n_t[:], zd[:])

            # state output: transpose h_T(127)
            pst = psum_s.tile([128, 128], F32, tag="ps")
            nc.tensor.transpose(pst[:B, :], outT[:, (S - 1) * B:S * B], ident_f[:])
            st_sb = work.tile([B, H], F32, tag="st_sb")
            nc.vector.tensor_copy(st_sb[:], pst[:B, :])
            nc.sync.dma_start(state_d[:], st_sb[:])

            # ---- P4/P5 per decoder block: attention then logits ---------
            for blk in range(NB):
                tlo = blk * BT                      # first timestep of block
                # bf16 cast of this block's GRU outputs
                nc.vector.tensor_copy(
                    outT_bf[:, tlo * B:(tlo + BT) * B],
                    outT[:, tlo * B:(tlo + BT) * B])

                for b in range(B):
                    # s^T[d, e] for d in block, batch b
                    lhs_cols = outT[:, tlo * B + b::B][:, 0:BT]
                    ps_s = psum_s.tile([BT, SE], F32, tag="ps")
                    nc.tensor.matmul(ps_s[:], lhs_cols,
                                     hsT[:, b * SE:(b + 1) * SE],
                                     start=True, stop=True)
                    nmax = grup.tile([BT, 1], F32, tag="nmax")
                    nc.vector.tensor_reduce(nmax[:], ps_s[:],
                                            axis=mybir.AxisListType.X,
                                            op=mybir.AluOpType.max, negate=True)
                    awT = grup.tile([BT, SE], F32, tag="awT")
                    sume = grup.tile([BT, 1], F32, tag="sume")
                    nc.scalar.activation(awT[:], ps_s[:],
                                         mybir.ActivationFunctionType.Exp,
                                         bias=nmax[:, 0:1], scale=1.0,
                                         accum_out=sume[:, 0:1])
                    rcp = grup.tile([BT, 1], F32, tag="rcp")
                    nc.vector.reciprocal(rcp[:], sume[:])
                    nc.vector.tensor_scalar_mul(awT[:], awT[:], rcp[:, 0:1])
                    # transpose -> aw[e, d]
                    pawt = psum_s.tile([SE, BT], F32, tag="ps")
                    nc.tensor.transpose(pawt[:], awT[:], ident_f[:BT, :BT])
                    aw_sb = grup.tile([SE, BT], F32, tag="aw_sb")
                    nc.vector.tensor_copy(aw_sb[:], pawt[:])
                    nc.sync.dma_start(aw_d[b, :, tlo:tlo + BT], aw_sb[:])
                    aw_bf = grup.tile([SE, BT], BF16, tag="aw_bf")
                    nc.scalar.copy(aw_bf[:], pawt[:])
                    # context^T[h, d] = hs[b]^T... lhsT=hs_bf (e,h), rhs=aw_bf
                    pc = psum_s.tile([128, BT], F32, tag="ps")
                    nc.tensor.matmul(pc[:], hs_bf[:, b * H:(b + 1) * H], aw_bf[:],
                                     start=True, stop=True)
                    nc.scalar.copy(cT_bf[:, tlo * B + b::B][:, 0:BT], pc[:])

                # logits for this block: 8 m-tiles of 128 toks
                for mi in range(BT * B // 128):
                    m0 = tlo * B + mi * 128      # token offset
                    t0 = tlo + mi * 4            # 4 timesteps per m-tile
                    for half in range(2):
                        pls = []
                        for nn in range(4):
                            n = half * 4 + nn
                            pl = psum_l.tile([128, VC], F32, tag="pl")
                            nc.tensor.matmul(
                                pl[:], outT_bf[:, m0:m0 + 128],
                                w_outT_bf[0][:, n * VC:(n + 1) * VC],
                                start=True, stop=False)
                            pls.append(pl)
                        for nn in range(4):
                            n = half * 4 + nn
                            nc.tensor.matmul(
                                pls[nn][:], cT_bf[:, m0:m0 + 128],
                                w_outT_bf[1][:, n * VC:(n + 1) * VC],
                                start=False, stop=True)
                        for nn in range(4):
                            n = half * 4 + nn
                            stg = stagep.tile([128, VC], F32, tag="stg")
                            nc.vector.tensor_add(
                                stg[:], pls[nn][:],
                                bcast_b[:, n * VC:(n + 1) * VC])
                            nc.sync.dma_start(
                                logits_d[t0:t0 + 4, :,
                                         n * VC:(n + 1) * VC].rearrange(
                                             "t b n -> (t b) n"),
                                stg[:])
    nc.finalize()
    return nc


_NC_CACHE = None
LAST_RESULT = None


def kernel(sequence, hs, h, emb, w_ih, w_hh, b_ih, b_hh, w_out, b_out):
    global _NC_CACHE
    sequence = np.asarray(sequence)
    hs = np.ascontiguousarray(np.asarray(hs, dtype=np.float32))
    h = np.ascontiguousarray(np.asarray(h, dtype=np.float32))
    emb = np.ascontiguousarray(np.asarray(emb, dtype=np.float32))
    w_ih = np.ascontiguousarray(np.asarray(w_ih, dtype=np.float32))
    w_hh = np.ascontiguousarray(np.asarray(w_hh, dtype=np.float32))
    b_ih = np.ascontiguousarray(np.asarray(b_ih, dtype=np.float32))
    b_hh = np.ascontiguousarray(np.asarray(b_hh, dtype=np.float32))
    w_out = np.asarray(w_out, dtype=np.float32)
    b_out = np.asarray(b_out, dtype=np.float32)

    seq_t = np.ascontiguousarray(sequence.T).reshape(-1).astype(np.int32)

    if _NC_CACHE is None:
        _NC_CACHE = build_nc()
    nc = _NC_CACHE

    in_maps = []
    for c in range(NCORES):
        in_maps.append({
            "seq_t": seq_t,
            "hs": hs,
            "h0": h[0],
            "emb": emb,
            "w_ih": w_ih,
            "w_hh": w_hh,
            "b_ih": b_ih,
            "b_hh": b_hh,
            "w_outT": np.ascontiguousarray(w_out[c * VS:(c + 1) * VS].T),
            "b_out": np.ascontiguousarray(b_out[c * VS:(c + 1) * VS]),
        })

    import os
    trace = bool(os.environ.get("ATT_DEC_TRACE"))
    res = run_bass_kernel_spmd(nc, in_maps, core_ids=list(range(NCORES)),
                               trace=trace)
    global LAST_RESULT
    LAST_RESULT = res
    results = res.results

    logits = np.empty((B, S, V), dtype=np.float32)
    for c in range(NCORES):
        logits[:, :, c * VS:(c + 1) * VS] = results[c]["logits"].transpose(1, 0, 2)
    state = results[0]["state"][None]
    aw = results[0]["aw"]
    return logits, state, aw


# revision 23
# speedup vs baseline: 1.0507x; 1.0507x over previous
"""AttentionDecoder kernel for 8 TRN2 NeuronCores.

Strategy (vocab-tensor-parallel, zero collectives):
  The GRU/attention part is tiny and latency-bound, so every core computes it
  redundantly (replicated) for the full batch; the heavy vocab projection
  (logits = cat @ w_out.T + b_out, 524 MB of output) is sharded over the vocab
  dimension: core c computes logits[:, :, c*4000:(c+1)*4000].  The host slices
  w_out/b_out per core and concatenates the logits shards; state/attention come
  from core 0.

Shapes (hardcoded): B=32, S_dec=128, S_enc=128, V=32000, E=200, H=128.
"""

import numpy as np

import concourse.bass as bass
from concourse import bacc
import concourse.mybir as mybir
import concourse.tile as tile
from concourse.bass import IndirectOffsetOnAxis
from concourse.bass_utils import run_bass_kernel_spmd
from concourse.masks import make_identity

B = 32
S = 128          # decoder length
SE = 128         # encoder length
EMB = 200        # embedding dim
H = 128          # hidden
V = 32000
NCORES = 8
VS = V // NCORES  # vocab shard = 4000
TOKS = B * S      # 4096, token index j = t*B + b  (time-major)

F32 = mybir.dt.float32
BF16 = mybir.dt.bfloat16
I32 = mybir.dt.int32

NB = 4            # decoder blocks
BT = S // NB      # timesteps per block = 32
NV = 8            # vocab chunks per core
VC = VS // NV     # 500 columns per chunk


def build_nc():
    nc = bacc.Bacc()

    # ---- I/O -------------------------------------------------------------
    seq_t = nc.declare_dram_parameter("seq_t", [TOKS], I32, isOutput=False)
    hs_d = nc.declare_dram_parameter("hs", [B, SE, H], F32, isOutput=False)
    h0_d = nc.declare_dram_parameter("h0", [B, H], F32, isOutput=False)
    emb_d = nc.declare_dram_parameter("emb", [V, EMB], F32, isOutput=False)
    w_ih_d = nc.declare_dram_parameter("w_ih", [3 * H, EMB], F32, isOutput=False)
    w_hh_d = nc.declare_dram_parameter("w_hh", [3 * H, H], F32, isOutput=False)
    b_ih_d = nc.declare_dram_parameter("b_ih", [3 * H], F32, isOutput=False)
    b_hh_d = nc.declare_dram_parameter("b_hh", [3 * H], F32, isOutput=False)
    w_outT_d = nc.declare_dram_parameter("w_outT", [2 * H, VS], F32, isOutput=False)
    b_out_d = nc.declare_dram_parameter("b_out", [VS], F32, isOutput=False)

    logits_d = nc.declare_dram_parameter("logits", [S, B, VS], F32, isOutput=True)
    state_d = nc.declare_dram_parameter("state", [B, H], F32, isOutput=True)
    aw_d = nc.declare_dram_parameter("aw", [B, SE, S], F32, isOutput=True)

    with tile.TileContext(nc) as tc:
        import contextlib
        ctx = contextlib.ExitStack()
        with ctx:
            persist = ctx.enter_context(tc.tile_pool(name="persist", bufs=1))
            work = ctx.enter_context(tc.tile_pool(name="work", bufs=3))
            stagep = ctx.enter_context(tc.tile_pool(name="stagep", bufs=4))
            grup = ctx.enter_context(tc.tile_pool(name="grup", bufs=3))
            psum_s = ctx.enter_context(
                tc.tile_pool(name="psum_s", bufs=3, space="PSUM"))
            psum_l = ctx.enter_context(
                tc.tile_pool(name="psum_l", bufs=3, space="PSUM"))

            # ---- persistent SBUF tensors --------------------------------
            ident_f = persist.tile([128, 128], F32)
            make_identity(nc, ident_f)

            xgT_rz = persist.tile([128, 2 * TOKS], F32)  # col = t*64 + g*32 + b
            xgT_n = persist.tile([128, TOKS], F32)       # col = t*32 + b
            outT = persist.tile([128, TOKS], F32)        # GRU outputs^T, col=t*32+b
            outT_bf = persist.tile([128, TOKS], BF16)
            cT_bf = persist.tile([128, TOKS], BF16)      # context^T, col=t*32+b
            hsT = persist.tile([128, B * SE], F32)       # hs[b]^T, col = b*128+e
            hs_bf = persist.tile([128, B * SE], BF16)    # hs[b],  col = b*128+h
            w_outT_bf = [persist.tile([128, VS], BF16, name=f"w_outT_bf{k}")
                         for k in range(2)]
            bcast_b = persist.tile([128, VS], BF16)      # b_out broadcast to rows
            hT0 = persist.tile([128, B], F32)
            ones_bf = persist.tile([1, 128], BF16)
            nc.gpsimd.memset(ones_bf[:], 1.0)

            w_ihT = [[persist.tile([128, 128], F32, name=f"w_ihT_{g}_{k}")
                      for k in range(2)] for g in range(3)]
            w_hhT = [persist.tile([128, 128], F32, name=f"w_hhT_{g}")
                     for g in range(3)]
            bias_r = persist.tile([128, 1], F32)
            bias_z = persist.tile([128, 1], F32)
            bih3 = persist.tile([128, 3], F32)
            bhh3 = persist.tile([128, 3], F32)
            bih3c = persist.tile([128, 3], F32)
            seq_sb = persist.tile([128, TOKS // 128], I32)

            # ---- P0: constants ------------------------------------------
            nc.sync.dma_start(seq_sb[:], seq_t.rearrange("(n p) -> p n", p=128))

            # biases: one DMA per tensor; each compute op waits on one queue
            nc.sync.dma_start(bih3[:], b_ih_d.rearrange("(g p) -> p g", p=128))
            nc.sync.dma_start(bhh3[:], b_hh_d.rearrange("(g p) -> p g", p=128))
            nc.vector.tensor_copy(bih3c[:], bih3[:])
            nc.vector.tensor_add(bias_r[:], bih3c[:, 0:1], bhh3[:, 0:1])
            nc.vector.tensor_add(bias_z[:], bih3c[:, 1:2], bhh3[:, 1:2])
            bias_nx = bih3c[:, 2:3]
            bias_nh = bhh3[:, 2:3]

            # w_ih -> w_ihT (bf16), w_hh -> w_hhT (f32)
            for g in range(3):
                wt = work.tile([128, EMB], F32, tag="wt")
                nc.sync.dma_start(wt[:], w_ih_d[g * 128:(g + 1) * 128, :])
                for k in range(2):
                    kn = 128 if k == 0 else EMB - 128
                    pt = psum_s.tile([128, 128], F32, tag="ps")
                    nc.tensor.transpose(
                        pt[:kn, :], wt[:, k * 128:k * 128 + kn], ident_f[:])
                    nc.vector.tensor_copy(w_ihT[g][k][:kn, :], pt[:kn, :])

                wh = work.tile([128, H], F32, tag="wh")
                nc.sync.dma_start(wh[:], w_hh_d[g * 128:(g + 1) * 128, :])
                ph = psum_s.tile([128, 128], F32, tag="ps")
                nc.tensor.transpose(ph[:], wh[:], ident_f[:])
                nc.vector.tensor_copy(w_hhT[g][:], ph[:])

            # h0 -> hT0
            h0t = work.tile([B, H], F32, tag="h0t")
            nc.sync.dma_start(h0t[:], h0_d[:])
            ph0 = psum_s.tile([128, 128], F32, tag="ps")
            nc.tensor.transpose(ph0[:, :B], h0t[:], ident_f[:B, :B])
            nc.vector.tensor_copy(hT0[:], ph0[:, :B])

            # hs: natural (bf16) + transposed (f32)
            for b in range(B):
                ht = work.tile([SE, H], F32, tag="ht")
                nc.sync.dma_start(ht[:], hs_d[b])
                nc.vector.tensor_copy(hs_bf[:, b * H:(b + 1) * H], ht[:])
                pht = psum_s.tile([128, 128], F32, tag="ps")
                nc.tensor.transpose(pht[:], ht[:], ident_f[:])
                nc.scalar.copy(hsT[:, b * SE:(b + 1) * SE], pht[:])

            # w_outT -> bf16; b_out -> broadcast rows (via ones matmul)
            bo = work.tile([1, VS], F32, tag="bo", bufs=1)
            nc.sync.dma_start(bo[:], b_out_d[None, :])
            bo_bf = work.tile([1, VS], BF16, tag="bo_bf", bufs=1)
            nc.vector.tensor_copy(bo_bf[:], bo[:])
            for k in range(2):
                for n in range(NV):
                    wo = stagep.tile([128, VC], F32, tag="wo", bufs=2)
                    nc.sync.dma_start(
                        wo[:], w_outT_d[k * 128:(k + 1) * 128,
                                        n * VC:(n + 1) * VC])
                    nc.vector.tensor_copy(
                        w_outT_bf[k][:, n * VC:(n + 1) * VC], wo[:])
            for n in range(NV):
                pb = psum_l.tile([128, VC], F32, tag="pl")
                nc.tensor.matmul(pb[:], ones_bf[0:1, :], bo_bf[0:1, n * VC:(n + 1) * VC],
                                 start=True, stop=True)
                nc.scalar.copy(bcast_b[:, n * VC:(n + 1) * VC], pb[:])

            # ---- P1+P2: embedding gather -> x^T (f32) -> x_gates^T ------
            xg_rz_4d = xgT_rz.rearrange("p (t g b) -> p t g b", g=2, b=B)
            for j in range(TOKS // 512):
                xt0 = work.tile([128, 512], F32, tag="xt0", bufs=2)
                xt1 = work.tile([128, 512], F32, tag="xt1", bufs=2)
                for i in range(4):
                    n = j * 4 + i
                    xr = work.tile([128, EMB], F32, tag="xr")
                    nc.gpsimd.indirect_dma_start(
                        out=xr[:], out_offset=None, in_=emb_d[:],
                        in_offset=IndirectOffsetOnAxis(
                            ap=seq_sb[:, n:n + 1], axis=0))
                    pt0 = psum_s.tile([128, 128], F32, tag="ps")
                    nc.tensor.transpose(pt0[:], xr[:, 0:128], ident_f[:])
                    nc.scalar.copy(xt0[:, i * 128:(i + 1) * 128], pt0[:])
                    pt1 = psum_s.tile([128, 128], F32, tag="ps")
                    nc.tensor.transpose(pt1[:EMB - 128, :], xr[:, 128:EMB],
                                        ident_f[:])
                    nc.scalar.copy(xt1[:EMB - 128, i * 128:(i + 1) * 128],
                                   pt1[:EMB - 128, :])
                t0 = j * 16   # 512 toks = 16 timesteps
                for g in range(3):
                    pxg = psum_s.tile([128, 512], F32, tag="ps")
                    nc.tensor.matmul(pxg[:], w_ihT[g][0][:], xt0[:],
                                     start=True, stop=False)
                    nc.tensor.matmul(pxg[:], w_ihT[g][1][:EMB - 128, :],
                                     xt1[:EMB - 128, :],
                                     start=False, stop=True)
                    if g == 0:
                        dst = xg_rz_4d[:, t0:t0 + 16, 0, :]
                        bias = bias_r
                    elif g == 1:
                        dst = xg_rz_4d[:, t0:t0 + 16, 1, :]
                        bias = bias_z
                    else:
                        dst = xgT_n[:, j * 512:(j + 1) * 512]
                        bias = bias_nx
                    nc.scalar.activation(
                        dst, pxg[:], mybir.ActivationFunctionType.Identity,
                        bias=bias[:, 0:1], scale=1.0)

            # ---- P3: GRU scan -------------------------------------------
            for t in range(S):
                hprev = hT0[:] if t == 0 else outT[:, (t - 1) * B:t * B]
                prz = psum_s.tile([128, 2 * B], F32, tag="psg", bufs=2)
                pn = psum_s.tile([128, B], F32, tag="psg", bufs=2)
                nc.tensor.matmul(prz[:, 0:B], w_hhT[0][:], hprev,
                                 start=True, stop=True)
                nc.tensor.matmul(prz[:, B:2 * B], w_hhT[1][:], hprev,
                                 start=True, stop=True)
                nc.tensor.matmul(pn[:], w_hhT[2][:], hprev, start=True, stop=True)

                rz_pre = grup.tile([128, 2 * B], F32, tag="rz_pre")
                nc.vector.tensor_add(rz_pre[:], xgT_rz[:, t * 2 * B:(t + 1) * 2 * B],
                                     prz[:])
                rz_sig = grup.tile([128, 2 * B], F32, tag="rz_sig")
                nc.scalar.activation(rz_sig[:], rz_pre[:],
                                     mybir.ActivationFunctionType.Sigmoid)
                hn_b = grup.tile([128, B], F32, tag="hn_b")
                nc.scalar.activation(hn_b[:], pn[:],
                                     mybir.ActivationFunctionType.Identity,
                                     bias=bias_nh[:, 0:1], scale=1.0)
                nr = grup.tile([128, B], F32, tag="nr")
                nc.vector.tensor_mul(nr[:], rz_sig[:, 0:B], hn_b[:])
                npre = grup.tile([128, B], F32, tag="npre")
                nc.vector.tensor_add(npre[:], nr[:], xgT_n[:, t * B:(t + 1) * B])
                n_t = grup.tile([128, B], F32, tag="n_t")
                nc.scalar.activation(n_t[:], npre[:],
                                     mybir.ActivationFunctionType.Tanh)
                d_t = grup.tile([128, B], F32, tag="d_t")
                nc.vector.tensor_sub(d_t[:], hprev, n_t[:])
                zd = grup.tile([128, B], F32, tag="zd")
                nc.vector.tensor_mul(zd[:], rz_sig[:, B:2 * B], d_t[:])
                nc.vector.tensor_add(outT[:, t * B:(t + 1) * B], n_t[:], zd[:])

            # state output: transpose h_T(127)
            pst = psum_s.tile([128, 128], F32, tag="ps")
            nc.tensor.transpose(pst[:B, :], outT[:, (S - 1) * B:S * B], ident_f[:])
            st_sb = work.tile([B, H], F32, tag="st_sb")
            nc.vector.tensor_copy(st_sb[:], pst[:B, :])
            nc.sync.dma_start(state_d[:], st_sb[:])

            # ---- P4/P5 per decoder block: attention then logits ---------
            for blk in range(NB):
                tlo = blk * BT                      # first timestep of block
                # bf16 cast of this block's GRU outputs
                nc.vector.tensor_copy(
                    outT_bf[:, tlo * B:(tlo + BT) * B],
                    outT[:, tlo * B:(tlo + BT) * B])

                for b in range(B):
                    # s^T[d, e] for d in block, batch b
                    lhs_cols = outT[:, tlo * B + b::B][:, 0:BT]
                    ps_s = psum_s.tile([BT, SE], F32, tag="ps")
                    nc.tensor.matmul(ps_s[:], lhs_cols,
                                     hsT[:, b * SE:(b + 1) * SE],
                                     start=True, stop=True)
                    nmax = grup.tile([BT, 1], F32, tag="nmax")
                    nc.vector.tensor_reduce(nmax[:], ps_s[:],
                                            axis=mybir.AxisListType.X,
                                            op=mybir.AluOpType.max, negate=True)
                    awT = grup.tile([BT, SE], F32, tag="awT")
                    sume = grup.tile([BT, 1], F32, tag="sume")
                    nc.scalar.activation(awT[:], ps_s[:],
                                         mybir.ActivationFunctionType.Exp,
                                         bias=nmax[:, 0:1], scale=1.0,
                                         accum_out=sume[:, 0:1])
                    rcp = grup.tile([BT, 1], F32, tag="rcp")
                    nc.vector.reciprocal(rcp[:], sume[:])
                    nc.vector.tensor_scalar_mul(awT[:], awT[:], rcp[:, 0:1])
                    # transpose -> aw[e, d]
                    pawt = psum_s.tile([SE, BT], F32, tag="ps")
                    nc.tensor.transpose(pawt[:], awT[:], ident_f[:BT, :BT])
                    aw_sb = grup.tile([SE, BT], F32, tag="aw_sb")
                    nc.vector.tensor_copy(aw_sb[:], pawt[:])
                    nc.sync.dma_start(aw_d[b, :, tlo:tlo + BT], aw_sb[:])
                    aw_bf = grup.tile([SE, BT], BF16, tag="aw_bf")
                    nc.scalar.copy(aw_bf[:], pawt[:])
                    # context^T[h, d] = hs[b]^T... lhsT=hs_bf (e,h), rhs=aw_bf
                    pc = psum_s.tile([128, BT], F32, tag="ps")
                    nc.tensor.matmul(pc[:], hs_bf[:, b * H:(b + 1) * H], aw_bf[:],
                                     start=True, stop=True)
                    nc.scalar.copy(cT_bf[:, tlo * B + b::B][:, 0:BT], pc[:])

                # logits for this block: 8 m-tiles of 128 toks
                for mi in range(BT * B // 128):
                    m0 = tlo * B + mi * 128      # token offset
                    t0 = tlo + mi * 4            # 4 timesteps per m-tile
                    for n in range(NV):
                        pl = psum_l.tile([128, VC], F32, tag="pl")
                        nc.tensor.matmul(
                            pl[:], outT_bf[:, m0:m0 + 128],
                            w_outT_bf[0][:, n * VC:(n + 1) * VC],
                            start=True, stop=False)
                        nc.tensor.matmul(
                            pl[:], cT_bf[:, m0:m0 + 128],
                            w_outT_bf[1][:, n * VC:(n + 1) * VC],
                            start=False, stop=True)
                        stg = stagep.tile([128, VC], F32, tag="stg")
                        nc.vector.tensor_add(
                            stg[:], pl[:],
                            bcast_b[:, n * VC:(n + 1) * VC])
                        nc.sync.dma_start(
                            logits_d[t0:t0 + 4, :,
                                     n * VC:(n + 1) * VC].rearrange(
                                         "t b n -> (t b) n"),
                            stg[:])
    nc.finalize()
    return nc


_NC_CACHE = None
LAST_RESULT = None


def kernel(sequence, hs, h, emb, w_ih, w_hh, b_ih, b_hh, w_out, b_out):
    global _NC_CACHE
    sequence = np.asarray(sequence)
    hs = np.ascontiguousarray(np.asarray(hs, dtype=np.float32))
    h = np.ascontiguousarray(np.asarray(h, dtype=np.float32))
    emb = np.ascontiguousarray(np.asarray(emb, dtype=np.float32))
    w_ih = np.ascontiguousarray(np.asarray(w_ih, dtype=np.float32))
    w_hh = np.ascontiguousarray(np.asarray(w_hh, dtype=np.float32))
    b_ih = np.ascontiguousarray(np.asarray(b_ih, dtype=np.float32))
    b_hh = np.ascontiguousarray(np.asarray(b_hh, dtype=np.float32))
    w_out = np.asarray(w_out, dtype=np.float32)
    b_out = np.asarray(b_out, dtype=np.float32)

    seq_t = np.ascontiguousarray(sequence.T).reshape(-1).astype(np.int32)

    if _NC_CACHE is None:
        _NC_CACHE = build_nc()
    nc = _NC_CACHE

    in_maps = []
    for c in range(NCORES):
        in_maps.append({
            "seq_t": seq_t,
            "hs": hs,
            "h0": h[0],
            "emb": emb,
            "w_ih": w_ih,
            "w_hh": w_hh,
            "b_ih": b_ih,
            "b_hh": b_hh,
            "w_outT": np.ascontiguousarray(w_out[c * VS:(c + 1) * VS].T),
            "b_out": np.ascontiguousarray(b_out[c * VS:(c + 1) * VS]),
        })

    import os
    trace = bool(os.environ.get("ATT_DEC_TRACE"))
    res = run_bass_kernel_spmd(nc, in_maps, core_ids=list(range(NCORES)),
                               trace=trace)
    global LAST_RESULT
    LAST_RESULT = res
    results = res.results

    logits = np.empty((B, S, V), dtype=np.float32)
    for c in range(NCORES):
        logits[:, :, c * VS:(c + 1) * VS] = results[c]["logits"].transpose(1, 0, 2)
    state = results[0]["state"][None]
    aw = results[0]["aw"]
    return logits, state, aw
